# revision 1
# baseline (speedup 1.0000x reference)
"""Trainium2 Bass kernel for nn_Decoder_46660524704357.

Reference computation (shapes hardcoded in DEFAULT_CFG):
    B, C, L, D, E, K = 64, 23, 26000, 64, 512, 3
    eos  = eos_emb @ eos_W.T + eos_b          # [B,C,D]
    bin_emb = emb_table[bin_ids]              # [C,L,D]
    a = bin_emb @ Wb.T                        # [C,L,K]   Wb = fc_W[:, :D]
    e = eos @ We.T + fc_b                     # [B,C,K]   We = fc_W[:, D:]
    out = relu(a[None,:,:,:] + e[:,:,None,:]) # [B,C,L,K]

Sharding: split L across the 8 cores (Lc = 3250 each).  Each core:
  - computes the eos projection chain (tiny) to get e[B,C,K] on-device,
  - for each chromosome c and each output tile, runs ONE fused matmul:
        out[p=(b*K+k), l] = sum_d Wsel[d, p] * embT[d, l] + e_row[p] * 1
    where Wsel[d, b*K+k] = fc_W[k, d] (replicated on-device from a [D,K]
    load) and the (D+1)-th contract row of embT is all-ones so the e term
    rides along.  ScalarE/VectorE apply ReLU on the PSUM->SBUF copy (bf16),
    DMA writes a [B, C, K, Lc] bf16 output.  Host upcasts to fp32 and
    re-interleaves K innermost at the end.

All large tensors ride in bf16 (embT read 9.7MB, out write 28.7MB per core
instead of 19.4/57.4 fp32) — the rel-err budget (2e-2) dwarfs bf16's ~0.4%.

Schedule notes (from NTFF traces):
  - et (embT slice) prefetch rides the otherwise-idle sync HWDGE queue and
    is paced by the emb pool's buffer semaphores, so it cannot crowd out
    the critical eosE/selector loads on the gpsimd queue at startup.
  - e_row is produced in 2-chromosome groups interleaved into the main
    loop, so chromosome 0's matmuls are unblocked ~1us after X is ready
    instead of waiting for the whole e computation.
  - PSUM out tiles span 3 banks (1536 cols): 3x fewer ReLU-eviction
    instructions (the +352cyc ACT / DVE per-instruction overhead was ~90us
    of engine time at 512-col granularity).
"""

import numpy as np
import ml_dtypes

BF16 = ml_dtypes.bfloat16

DEFAULT_CFG = dict(B=64, C=23, L=26000, D=64, E=512, K=3, NCORES=8)

_CACHE = {}


def _derived(cfg):
    B, C, L, D, E, K, NCORES = (cfg[k] for k in ("B", "C", "L", "D", "E", "K", "NCORES"))
    d = dict(cfg)
    d["LC"] = L // NCORES
    d["BC"] = B * C
    d["EP"] = min(128, E)              # contract chunk for eos matmul
    assert E % d["EP"] == 0
    d["NQ"] = E // d["EP"]
    d["ROWS"] = K * B                  # output partition rows (b*K + k)
    # partition tiles over ROWS: cut at b boundaries so each tile's DMA rows
    # merge into contiguous [K*LC] runs per b
    tiles = []
    bmax = 128 // K                    # b's per tile
    b0 = 0
    while b0 < B:
        nb = min(bmax, B - b0)
        tiles.append((b0 * K, nb * K, b0, nb))
        b0 += nb
    d["PTILES"] = tiles                # (p_off, p_n, b0, nb)
    # PSUM out tiles: 2 banks (1024 f32 cols) each, filled by <=512-col matmuls
    big = 1024
    d["NF"] = [big] * (d["LC"] // big) + ([d["LC"] % big] if d["LC"] % big else [])
    d["MM"] = 512                      # matmul moving-operand chunk
    d["EGRP"] = 2                      # chromosomes per e_row colsum group
    d["PREF"] = 12                     # emb pool depth (prefetch pacing)
    return d


def _build_nc(cfg=None):
    import concourse.bass as bass  # noqa: F401
    import concourse.mybir as mybir
    import concourse.tile as tile
    from concourse import bacc

    g = _derived(cfg or DEFAULT_CFG)
    B, C, D, K = g["B"], g["C"], g["D"], g["K"]
    LC, BC, EP, NQ, ROWS = g["LC"], g["BC"], g["EP"], g["NQ"], g["ROWS"]
    MM, EGRP = g["MM"], g["EGRP"]
    FCH = min(512, BC)

    f32 = mybir.dt.float32
    bf16 = mybir.dt.bfloat16
    fsel = bf16   # selector matmul operands (embT stream + wsel stationary)
    feos = bf16   # eos projection matmul operands (eosE stream + eosW stationary)

    # Bacc (not plain Bass): its compile() passes split multi-sem waits and
    # move matmul waits to ldweights — required for walrus codegen.
    nc = bacc.Bacc(None)

    embT = nc.declare_dram_parameter("embT", [D + 1, C * LC], fsel, isOutput=False)
    eosE = nc.declare_dram_parameter("eosE", [EP, NQ * BC], feos, isOutput=False)
    eosW = nc.declare_dram_parameter("eosW", [EP, NQ * D], feos, isOutput=False)
    WbT = nc.declare_dram_parameter("WbT", [D, K], fsel, isOutput=False)
    WeT = nc.declare_dram_parameter("WeT", [D, K], f32, isOutput=False)
    # eos_b carries We^+ @ fc_b folded in (host-side), so fc_b vanishes here
    eos_b = nc.declare_dram_parameter("eos_b", [D, 1], f32, isOutput=False)
    out = nc.declare_dram_parameter("out", [B, C, K, LC], bf16, isOutput=True)

    with tile.TileContext(nc) as tc:
        with (
            tc.tile_pool(name="consts", bufs=1) as consts,
            tc.tile_pool(name="setup_sb", bufs=1) as setup_sb,
            tc.tile_pool(name="emb", bufs=g["PREF"]) as emb_pool,
            tc.tile_pool(name="osb", bufs=8) as osb_pool,
            tc.tile_pool(name="ops", bufs=3, space="PSUM") as ops_pool,
            tc.tile_pool(name="eps", bufs=2, space="PSUM") as eps_pool,
        ):
            # ---- setup loads: eosE chunks split across all three DMA
            # queues so the eos chain's data lands in ~1/3 the time --------
            eosE_sb = setup_sb.tile([EP, NQ * BC], feos)
            eosE_engines = [nc.gpsimd, nc.scalar, nc.sync, nc.gpsimd]
            for q in range(NQ):
                eosE_engines[q % len(eosE_engines)].dma_start(
                    eosE_sb[:, q * BC:(q + 1) * BC],
                    eosE[:, q * BC:(q + 1) * BC])
            eosW_sb = setup_sb.tile([EP, NQ * D], feos)
            nc.gpsimd.dma_start(eosW_sb[:, :], eosW[:, :])
            WbT_sb = setup_sb.tile([D, K], fsel)
            nc.gpsimd.dma_start(WbT_sb[:, :], WbT[:, :])
            WeT_sb = setup_sb.tile([D, K], f32)
            nc.gpsimd.dma_start(WeT_sb[:, :], WeT[:, :])
            eosb_sb = setup_sb.tile([D, 1], f32)
            nc.gpsimd.dma_start(eosb_sb[:, :], eos_b[:, :])

            # selector weights: rows 0..D-1 replicated on-device from WbT
            # (saves the 0.55MB wsel load), row D = e_row (written below).
            # GpSimd does the copy — DVE is needed for X / evictions.
            se = consts.tile([D + 1, C * ROWS], fsel)
            se_w = se[0:D, :].rearrange("d (r k) -> d r k", r=C * B, k=K)
            WbT_g = WbT_sb[:, :].unsqueeze(1).broadcast_to([D, C * B, K])
            nc.gpsimd.tensor_copy(se_w, WbT_g)

            # PE warm-up: back-to-back junk matmuls from t~6us (no data
            # deps) push HAM to K=8/8 (2.4 GHz) while eosE is in flight.
            # A cold PE (1.2 GHz) costs ~60us over this kernel, and HAM
            # cannot re-warm mid-loop (needs ~3.4us of *contiguous* PE
            # activity, which the eviction-paced main loop never gives it).
            junk = setup_sb.tile([D, 512], bf16)
            nc.vector.memset(junk[:, :], 0.0)
            j_ps = eps_pool.tile([D + 1, 512], f32, tag="e_ps")
            for i in range(40):
                # one accumulation group: same engine, same bank, no
                # semaphores -> truly back-to-back (the pool-rotation form
                # had ~50% duty cycle and never tripped HAM's SHORT window)
                nc.tensor.matmul(
                    j_ps[0:1, :], lhsT=junk[:, 0:1], rhs=junk[:, :],
                    start=(i == 0), stop=(i == 39),
                )

            # eosT[d, cb] = sum_E eos_W[d, E] * eos_emb[cb, E]  (+ eos_b)
            # (cb is c-major: cb = c*B + b — matches selector column order)
            eosT_sb = setup_sb.tile([D, BC], f32)
            bc_chunks = [(i, min(FCH, BC - i)) for i in range(0, BC, FCH)]
            for bc0, nbc in bc_chunks:
                eosT_ps = ops_pool.tile([D, nbc], f32, tag="out_ps")
                for q in range(NQ):
                    nc.tensor.matmul(
                        eosT_ps[:, :],
                        lhsT=eosW_sb[:, q * D:(q + 1) * D],
                        rhs=eosE_sb[:, q * BC + bc0: q * BC + bc0 + nbc],
                        start=(q == 0),
                        stop=(q == NQ - 1),
                    )
                nc.scalar.add(eosT_sb[:, bc0:bc0 + nbc], eosT_ps[:, :], eosb_sb[:, 0:1])

            # second junk batch: keeps PE busy across the eosT->X->colsum
            # handoff so HAM stays at K=8/8 into the main loop
            j_ps2 = eps_pool.tile([D + 1, 512], f32, tag="e_ps")
            for i in range(30):
                nc.tensor.matmul(
                    j_ps2[0:1, :], lhsT=junk[:, 0:1], rhs=junk[:, :],
                    start=(i == 0), stop=(i == 29),
                )

            # X[d, (c,b,k)] = eosT[d, c*B+b] * WeT[d, k]  (bf16 so the
            # per-group colsum matmuls stream at 1 col/cycle).  Computed
            # per-EGRP-group interleaved with the main loop so chromosome
            # 0's selector column is ready ~1us after eosT lands.
            X = setup_sb.tile([D, C * ROWS], bf16)
            eosT_g4 = eosT_sb[:, :].rearrange("d (c b) -> d c b", c=C, b=B) \
                .unsqueeze(3).broadcast_to([D, C, B, K])
            We_g = WeT_sb[:, :].unsqueeze(1).unsqueeze(1).broadcast_to([D, C, B, K])
            X_w4 = X[:, :].rearrange("d (c b k) -> d c b k", c=C, b=B, k=K)

            ones64 = setup_sb.tile([D, 1], bf16)
            nc.vector.memset(ones64[:, :], 1.0)
            sc_t = 0.0
            ve_t = 0.0

            def emit_e_rows(c0):
                """X group mul + colsum over d -> se[D, c0*ROWS : ...]."""
                ng = min(EGRP, C - c0)
                ncols = ng * ROWS
                nc.vector.tensor_mul(
                    X_w4[:, c0:c0 + ng], eosT_g4[:, c0:c0 + ng], We_g[:, c0:c0 + ng])
                e_ps = eps_pool.tile([D + 1, EGRP * ROWS], f32, tag="e_ps")
                nc.tensor.matmul(
                    e_ps[D:D + 1, 0:ncols],
                    lhsT=ones64[:, 0:1],
                    rhs=X[:, c0 * ROWS: c0 * ROWS + ncols],
                    start=True,
                    stop=True,
                )
                nc.scalar.activation(
                    se[D:D + 1, c0 * ROWS: c0 * ROWS + ncols],
                    e_ps[D:D + 1, 0:ncols],
                    mybir.ActivationFunctionType.Copy,
                )

            # ---- main loop ---------------------------------------------
            out_bkl = out.rearrange("b c k l -> c b (k l)")
            # out-DMAs ride gpsimd (SWDGE, 16-way descriptor fan-out) and
            # scalar (HWDGE); et loads ride sync so their pool-semaphore
            # pacing can't block an out trigger.
            out_engines = [nc.gpsimd, nc.gpsimd, nc.scalar, nc.gpsimd]
            n_dma = 0
            for c in range(C):
                if c % EGRP == 0:
                    emit_e_rows(c)
                et = emb_pool.tile([D + 1, LC], fsel, tag="embT")
                nc.sync.dma_start(et[:, :], embT[:, c * LC:(c + 1) * LC])
                for ti, (p_off, p_n, b0, nb) in enumerate(g["PTILES"]):
                    so = osb_pool.tile([p_n, LC], bf16, tag="out_sb")
                    f0 = 0
                    for nf in g["NF"]:
                        po = ops_pool.tile([p_n, nf], f32, tag="out_ps")
                        for m0 in range(0, nf, MM):
                            mn = min(MM, nf - m0)
                            nc.tensor.matmul(
                                po[:, m0:m0 + mn],
                                lhsT=se[:, c * ROWS + p_off: c * ROWS + p_off + p_n],
                                rhs=et[:, f0 + m0: f0 + m0 + mn],
                                start=True,
                                stop=True,
                            )
                        # ReLU on PSUM->SBUF copy, split by measured engine
                        # rates: ACT ~(nf+352)/1.2 ns, DVE ~1.15*nf ns
                        if sc_t + (nf + 352) * 0.833 <= ve_t + 1.15 * nf:
                            sc_t += (nf + 352) * 0.833
                            nc.scalar.activation(
                                so[:, f0:f0 + nf], po[:, :],
                                mybir.ActivationFunctionType.Relu,
                            )
                        else:
                            ve_t += 1.15 * nf
                            nc.vector.tensor_scalar_max(so[:, f0:f0 + nf], po[:, :], 0.0)
                        f0 += nf
                    out_engines[n_dma % len(out_engines)].dma_start(
                        out_bkl[c, b0:b0 + nb, :], so[:, :]
                    )
                    n_dma += 1
    nc.finalize()
    return nc


def _host_prep(eos_emb, bin_ids, emb_table, eos_W, eos_b, fc_W, fc_b, cfg=None):
    """Build the per-core input maps."""
    g = _derived(cfg or DEFAULT_CFG)
    B, C, L, D, E, K = g["B"], g["C"], g["L"], g["D"], g["E"], g["K"]
    NCORES, LC, BC, EP, NQ = g["NCORES"], g["LC"], g["BC"], g["EP"], g["NQ"]

    eos_emb = np.ascontiguousarray(eos_emb, dtype=np.float32)
    emb_table = np.ascontiguousarray(emb_table, dtype=np.float32)
    bin_ids = np.asarray(bin_ids)

    # gather (identity when bin_ids == arange, which is the spec'd fill)
    V = C * L
    flat_ids = bin_ids.reshape(-1)
    if flat_ids.shape[0] == V and emb_table.shape[0] == V and \
            flat_ids[0] == 0 and flat_ids[-1] == V - 1 and \
            np.array_equal(flat_ids, np.arange(V, dtype=flat_ids.dtype)):
        bin_emb = emb_table.reshape(C, L, D)
    else:
        bin_emb = emb_table[bin_ids.reshape(C, L)]

    # eosE[ep, q*BC + c*B + b] = eos_emb[b, c, q*EP + ep]   (c-major bc)
    eosE = np.ascontiguousarray(
        eos_emb.transpose(2, 1, 0).reshape(NQ, EP, BC).transpose(1, 0, 2).reshape(EP, NQ * BC)
    ).astype(BF16)
    eosW = np.ascontiguousarray(
        np.asarray(eos_W, np.float32).T.reshape(NQ, EP, D).transpose(1, 0, 2).reshape(EP, NQ * D)
    ).astype(BF16)
    fc_W = np.asarray(fc_W, np.float32)
    WbT = np.ascontiguousarray(fc_W[:, :D].T).astype(BF16)  # [D, K]
    WeT = np.ascontiguousarray(fc_W[:, D:].T)               # [D, K]
    # fold fc_b into eos_b: find delta with We @ delta == fc_b (min-norm
    # solution; exact since We [K,D] has full row rank), then
    # e = We @ (eos + delta) = We @ eos + fc_b.
    We64 = fc_W[:, D:].astype(np.float64)                   # [K, D]
    fcb64 = np.asarray(fc_b, np.float64).reshape(K)
    delta = We64.T @ np.linalg.solve(We64 @ We64.T, fcb64)  # [D]
    eos_b_in = (np.asarray(eos_b, np.float64).reshape(D) + delta) \
        .astype(np.float32).reshape(D, 1)

    shared = dict(eosE=eosE, eosW=eosW, WbT=WbT, WeT=WeT, eos_b=eos_b_in)

    in_maps = []
    for i in range(NCORES):
        sl = bin_emb[:, i * LC:(i + 1) * LC, :]          # [C, Lc, D]
        embT_i = np.empty((D + 1, C * LC), BF16)
        embT_i[:D] = sl.transpose(2, 0, 1).reshape(D, C * LC).astype(BF16)
        embT_i[D] = np.float32(1.0)
        in_maps.append({"embT": embT_i, **shared})
    return in_maps


def _assemble(results, cfg=None):
    g = _derived(cfg or DEFAULT_CFG)
    B, C, L, K, NCORES, LC = g["B"], g["C"], g["L"], g["K"], g["NCORES"], g["LC"]
    out = np.empty((B, C, L, K), np.float32)
    for i in range(NCORES):
        r = results[i]["out"]                            # [B, C, K, Lc] bf16
        out[:, :, i * LC:(i + 1) * LC, :] = r.transpose(0, 1, 3, 2)
    return out


def kernel(eos_emb, bin_ids, emb_table, eos_W, eos_b, fc_W, fc_b):
    from concourse.bass_utils import run_bass_kernel_spmd

    if "nc" not in _CACHE:
        _CACHE["nc"] = _build_nc()
    nc = _CACHE["nc"]
    in_maps = _host_prep(eos_emb, bin_ids, emb_table, eos_W, eos_b, fc_W, fc_b)
    res = run_bass_kernel_spmd(nc, in_maps, core_ids=list(range(DEFAULT_CFG["NCORES"])))
    return _assemble(res.results)



# revision 5
# speedup vs baseline: 1.6145x; 1.6145x over previous
"""Trainium2 Bass kernel for nn_Decoder_46660524704357.

Reference computation (shapes hardcoded in DEFAULT_CFG):
    B, C, L, D, E, K = 64, 23, 26000, 64, 512, 3
    eos  = eos_emb @ eos_W.T + eos_b          # [B,C,D]
    bin_emb = emb_table[bin_ids]              # [C,L,D]
    a = bin_emb @ Wb.T                        # [C,L,K]   Wb = fc_W[:, :D]
    e = eos @ We.T + fc_b                     # [B,C,K]   We = fc_W[:, D:]
    out = relu(a[None,:,:,:] + e[:,:,None,:]) # [B,C,L,K]

Sharding: split L across the 8 cores (Lc = 3250 each).

v2 design (vs the v1 selector-matmul kernel):
  - e[B,C,K] (4416 floats, 0.003% of the FLOPs) is computed on host in
    fp32 and shipped as per-partition bias columns.  The eviction engines
    fuse it into the ReLU on the PSUM->SBUF copy (ACT: activation(Relu,
    bias=e_col); DVE: tensor_scalar(add e_col, max 0)).  This removes the
    ones-row/X/colsum machinery and shrinks the matmul contract to D=64.
  - Contract 64 = half the PE array -> two chromosomes are processed by
    CONCURRENT row-group-tiled matmuls (array rows 0:64 stream c_even,
    rows 64:128 stream c_odd; tile_position auto-derived from base
    partitions).  Out tiles are 128 rows: (b,k) rows 0:128 for each c in
    step 1, and the two 64-row remainders merged into one 128-row PSUM
    tile in step 2 (tile_position (0,0) / (64,64)).  Streamed/evicted
    columns drop from 149.5k (v1: 126+66-row tiles) to 112k, and the PE
    covers 2 chromosomes per streamed column.
  - DMA routing: et pair loads ride sync (HWDGE) exclusively so prefetch
    is never head-of-line blocked; all out DMAs ride gpsimd (SWDGE),
    which has no other duty in this design.
  - Expected pacing: out-DMA 28.7MB + et 10MB at ~358 GB/s ~= 107us floor;
    eviction ~59us; PE ~65us even fully cold (1.2 GHz), so HAM state no
    longer matters.

All large tensors ride in bf16 (rel-err budget 2e-2 dwarfs bf16 ~0.4%).
"""

import numpy as np
import ml_dtypes

BF16 = ml_dtypes.bfloat16

DEFAULT_CFG = dict(B=64, C=23, L=26000, D=64, E=512, K=3, NCORES=8)

_CACHE = {}


def _derived(cfg):
    B, C, L, D, E, K, NCORES = (cfg[k] for k in ("B", "C", "L", "D", "E", "K", "NCORES"))
    d = dict(cfg)
    d["LC"] = L // NCORES
    d["ROWS"] = B * K                   # 192 output rows per chromosome
    d["NPAIR"] = C // 2                 # 11 full pairs, c=22 is a singleton
    d["NSLOT"] = d["NPAIR"] * 3 + 2     # bias columns (3 per pair, 2 singleton)
    big = 1024                          # bf16 moving-operand max
    d["NF"] = [big] * (d["LC"] // big) + ([d["LC"] % big] if d["LC"] % big else [])
    return d


def _build_nc(cfg=None):
    import concourse.bass as bass  # noqa: F401
    import concourse.mybir as mybir
    import concourse.tile as tile
    from concourse import bacc

    g = _derived(cfg or DEFAULT_CFG)
    C, D, LC = g["C"], g["D"], g["LC"]
    ROWS, NPAIR, NSLOT = g["ROWS"], g["NPAIR"], g["NSLOT"]

    f32 = mybir.dt.float32
    bf16 = mybir.dt.bfloat16

    nc = bacc.Bacc(None)

    # embT: pair p holds c=2p on rows 0:64, c=2p+1 on rows 64:128
    # (pair NPAIR = singleton c=C-1 on rows 0:64 only; rows 64:128 unused)
    embT = nc.declare_dram_parameter("embT", [2 * D, (NPAIR + 1) * LC], bf16, isOutput=False)
    # W2: rows 0:64 and 64:128 both hold Wrep[d, (b,k)] = Wb[k, d] tiled over b
    W2 = nc.declare_dram_parameter("W2", [2 * D, ROWS], bf16, isOutput=False)
    # ecol: per-eviction-tile per-partition bias columns (see _host_prep)
    ecol = nc.declare_dram_parameter("ecol", [128, NSLOT], f32, isOutput=False)
    out = nc.declare_dram_parameter("out", [C, ROWS, LC], bf16, isOutput=True)

    with tile.TileContext(nc) as tc:
        with (
            tc.tile_pool(name="consts", bufs=1) as consts,
            tc.tile_pool(name="emb", bufs=4) as emb_pool,
            tc.tile_pool(name="osb", bufs=6) as osb_pool,
            tc.tile_pool(name="ops", bufs=4, space="PSUM") as ops_pool,
        ):
            W2_sb = consts.tile([2 * D, ROWS], bf16)
            nc.scalar.dma_start(W2_sb[:, :], W2[:, :])
            ecol_sb = consts.tile([128, NSLOT], f32)
            nc.scalar.dma_start(ecol_sb[:, :], ecol[:, :])

            # eviction-engine balancer: ACT ~ nf*0.833 + 175ns, DVE ~ nf*1.042 + 170ns
            eng_t = [0.0, 0.0]  # ACT, DVE

            def evict(dst, src, bias_ap, nf):
                act_cost = nf * 0.833 + 175.0
                dve_cost = nf * 1.042 + 170.0
                if eng_t[0] + act_cost <= eng_t[1] + dve_cost:
                    eng_t[0] += act_cost
                    nc.scalar.activation(
                        dst, src, mybir.ActivationFunctionType.Relu, bias=bias_ap,
                    )
                else:
                    eng_t[1] += dve_cost
                    nc.vector.tensor_scalar(
                        dst, src, bias_ap, 0.0,
                        mybir.AluOpType.add, mybir.AluOpType.max,
                    )

            n_out_dma = 0

            def do_pair(p, paired):
                nonlocal n_out_dma
                c0 = 2 * p
                nrow = 2 * D if paired else D
                et = emb_pool.tile([2 * D, LC], bf16, tag="et")
                nc.sync.dma_start(et[0:nrow, :], embT[0:nrow, p * LC:(p + 1) * LC])

                soA = osb_pool.tile([128, LC], bf16, tag="so", name=f"soA_{p}")
                soB = (osb_pool.tile([128, LC], bf16, tag="so", name=f"soB_{p}")
                       if paired else None)
                so2 = osb_pool.tile([128, LC], bf16, tag="so", name=f"so2_{p}")

                sA, sB, s2 = 3 * p, 3 * p + 1, 3 * p + 2
                if not paired:
                    sA, s2 = 3 * p, 3 * p + 1

                # step 1: (b,k) rows 0:128 for each chromosome of the pair
                # (matmul output <= 1 PSUM bank = 512 f32 cols; evictions
                # cover the full 2-bank 1024-col tile in one instruction)
                f0 = 0
                for nf in g["NF"]:
                    psA = ops_pool.tile([128, 1024], f32, tag="ps")
                    psB = (ops_pool.tile([128, 1024], f32, tag="ps", name=f"psB_{p}_{f0}")
                           if paired else None)
                    for m0 in range(0, nf, 512):
                        mn = min(512, nf - m0)
                        nc.tensor.matmul(
                            psA[:, m0:m0 + mn], lhsT=W2_sb[0:D, 0:128],
                            rhs=et[0:D, f0 + m0:f0 + m0 + mn], start=True, stop=True,
                        )
                        if paired:
                            nc.tensor.matmul(
                                psB[:, m0:m0 + mn], lhsT=W2_sb[D:2 * D, 0:128],
                                rhs=et[D:2 * D, f0 + m0:f0 + m0 + mn],
                                start=True, stop=True,
                            )
                    evict(soA[:, f0:f0 + nf], psA[:, 0:nf], ecol_sb[:, sA:sA + 1], nf)
                    if paired:
                        evict(soB[:, f0:f0 + nf], psB[:, 0:nf], ecol_sb[:, sB:sB + 1], nf)
                    f0 += nf

                # step 2: rows 128:192 of both chromosomes, merged into one
                # 128-row PSUM tile (c_even -> partitions 0:64 via tile (0,0),
                # c_odd -> partitions 64:128 via tile (64,64))
                f0 = 0
                for nf in g["NF"]:
                    ps2 = ops_pool.tile([128, 1024], f32, tag="ps")
                    for m0 in range(0, nf, 512):
                        mn = min(512, nf - m0)
                        nc.tensor.matmul(
                            ps2[0:D, m0:m0 + mn], lhsT=W2_sb[0:D, 128:ROWS],
                            rhs=et[0:D, f0 + m0:f0 + m0 + mn], start=True, stop=True,
                        )
                        if paired:
                            nc.tensor.matmul(
                                ps2[D:128, m0:m0 + mn], lhsT=W2_sb[D:2 * D, 128:ROWS],
                                rhs=et[D:2 * D, f0 + m0:f0 + m0 + mn],
                                start=True, stop=True,
                            )
                    nrow2 = 128 if paired else D
                    evict(so2[0:nrow2, f0:f0 + nf], ps2[0:nrow2, 0:nf],
                          ecol_sb[0:nrow2, s2:s2 + 1], nf)
                    f0 += nf

                # out DMAs (SWDGE / gpsimd — free in this design)
                nc.gpsimd.dma_start(out[c0, 0:128, :], soA[:, :])
                if paired:
                    nc.gpsimd.dma_start(out[c0 + 1, 0:128, :], soB[:, :])
                nc.gpsimd.dma_start(out[c0, 128:ROWS, :], so2[0:D, :])
                if paired:
                    nc.gpsimd.dma_start(out[c0 + 1, 128:ROWS, :], so2[D:128, :])
                n_out_dma += 4 if paired else 2

            for p in range(NPAIR):
                do_pair(p, paired=True)
            do_pair(NPAIR, paired=False)
    nc.finalize()
    return nc


def _host_prep(eos_emb, bin_ids, emb_table, eos_W, eos_b, fc_W, fc_b, cfg=None):
    """Build the per-core input maps."""
    g = _derived(cfg or DEFAULT_CFG)
    B, C, L, D, E, K = g["B"], g["C"], g["L"], g["D"], g["E"], g["K"]
    NCORES, LC, ROWS, NPAIR, NSLOT = (
        g["NCORES"], g["LC"], g["ROWS"], g["NPAIR"], g["NSLOT"])

    eos_emb = np.ascontiguousarray(eos_emb, dtype=np.float32)
    emb_table = np.ascontiguousarray(emb_table, dtype=np.float32)
    bin_ids = np.asarray(bin_ids)
    fc_W = np.asarray(fc_W, np.float32)

    # gather (identity when bin_ids == arange, which is the spec'd fill)
    V = C * L
    flat_ids = bin_ids.reshape(-1)
    if flat_ids.shape[0] == V and emb_table.shape[0] == V and \
            flat_ids[0] == 0 and flat_ids[-1] == V - 1 and \
            np.array_equal(flat_ids, np.arange(V, dtype=flat_ids.dtype)):
        bin_emb = emb_table.reshape(C, L, D)
    else:
        bin_emb = emb_table[bin_ids.reshape(C, L)]

    # e[b,c,k] = (eos_emb[b,c] @ eos_W.T + eos_b) @ We.T + fc_b  (exact, f32)
    eos = np.einsum("bce,de->bcd", eos_emb, np.asarray(eos_W, np.float32),
                    optimize=True) + np.asarray(eos_b, np.float32)
    e = np.einsum("bcd,kd->bck", eos, fc_W[:, D:], optimize=True) \
        + np.asarray(fc_b, np.float32)
    ef = e.transpose(1, 0, 2).reshape(C, ROWS)      # [c, b*K+k]

    # ecol: bias column per eviction tile
    ecol = np.zeros((128, NSLOT), np.float32)
    for p in range(NPAIR):
        c0 = 2 * p
        ecol[:, 3 * p] = ef[c0, 0:128]
        ecol[:, 3 * p + 1] = ef[c0 + 1, 0:128]
        ecol[0:D, 3 * p + 2] = ef[c0, 128:ROWS]
        ecol[D:128, 3 * p + 2] = ef[c0 + 1, 128:ROWS]
    ecol[:, 3 * NPAIR] = ef[C - 1, 0:128]
    ecol[0:D, 3 * NPAIR + 1] = ef[C - 1, 128:ROWS]

    # W2: Wrep[d, (b,k)] = Wb[k,d], replicated on both row halves
    Wrep = np.ascontiguousarray(
        np.broadcast_to(fc_W[:, :D].T[:, None, :], (D, B, K)).reshape(D, ROWS))
    W2 = np.empty((2 * D, ROWS), np.float32)
    W2[0:D] = Wrep
    W2[D:2 * D] = Wrep
    W2 = W2.astype(BF16)

    shared = dict(W2=W2, ecol=ecol)

    in_maps = []
    for i in range(NCORES):
        sl = bin_emb[:, i * LC:(i + 1) * LC, :]          # [C, Lc, D]
        slT = sl.transpose(0, 2, 1)                      # [C, D, Lc]
        embT_i = np.zeros((2 * D, (NPAIR + 1) * LC), BF16)
        for p in range(NPAIR):
            embT_i[0:D, p * LC:(p + 1) * LC] = slT[2 * p].astype(BF16)
            embT_i[D:2 * D, p * LC:(p + 1) * LC] = slT[2 * p + 1].astype(BF16)
        embT_i[0:D, NPAIR * LC:] = slT[C - 1].astype(BF16)
        in_maps.append({"embT": embT_i, **shared})
    return in_maps


def _assemble(results, cfg=None):
    g = _derived(cfg or DEFAULT_CFG)
    B, C, L, K, NCORES, LC = g["B"], g["C"], g["L"], g["K"], g["NCORES"], g["LC"]
    out = np.empty((B, C, L, K), np.float32)
    for i in range(NCORES):
        r = np.asarray(results[i]["out"])                # [C, B*K, Lc] bf16
        r = r.reshape(C, B, K, LC)
        out[:, :, i * LC:(i + 1) * LC, :] = r.transpose(1, 0, 3, 2)
    return out


def kernel(eos_emb, bin_ids, emb_table, eos_W, eos_b, fc_W, fc_b):
    from concourse.bass_utils import run_bass_kernel_spmd

    if "nc" not in _CACHE:
        _CACHE["nc"] = _build_nc()
    nc = _CACHE["nc"]
    in_maps = _host_prep(eos_emb, bin_ids, emb_table, eos_W, eos_b, fc_W, fc_b)
    res = run_bass_kernel_spmd(nc, in_maps, core_ids=list(range(DEFAULT_CFG["NCORES"])))
    return _assemble(res.results)
